# revision 9
# baseline (speedup 1.0000x reference)
"""Trainium2 Bass kernel for nn_NestRQModel (NEST-RQ pretraining loss).

Strategy: pure data-parallel over batch (2 batches per core, 8 cores), no
collectives.  Each core computes partial masked sums (nll, corr) and a
vocab-presence bitmap; the host combines them into the 4 scalar outputs.

Per-core pipeline (1024 rows = 2 batches x 512 frames):
  stage 0: LayerNorm stats (bn_stats) on stacked frames [128,320/tile];
           projection matmul q = stack @ P with LN folded in afterwards as a
           per-row affine fix  x = s*(q - mu*colsum(P))  (LN commutes through
           the linear projection).  Row-norm of x is skipped entirely: codes
           are an argmax over cosine-monotone scores, and the e2 term of the
           reference distance varies by < 1e-8 across the codebook.
  stage A: dots = x @ E^T as ONE K=64 matmul per tile via bf16 hi/lo Kahan
           stacking (error ~1e-6, full PE rate); argmax via DVE max/max_index.
  stage B: logits = enc @ W in float32r (full PE rate, N=512);
           ACT Exp with accum_out -> row sum-of-exp (no max subtraction
           needed: |logits| < ~4 so exp cannot overflow);
           corr  = [#(exp(logits) > exp(tgt+margin)) == 0] via one 2x-mode
           tensor_scalar is_gt with accum;
           tgt logit = indirect-DMA gather of W^T rows + fused TT-reduce dot.
  presence: indirect-DMA scatter of 1.0 at (code*mask) indices — identical
           semantics to reference's presence[masked_tgt]=1 (masked rows
           scatter index 0).
"""
import os
import sys

import numpy as np

os.environ.setdefault("MYCRO_LOCAL_CACHE", "1")

try:
    import concourse.bass as bass
except ImportError:
    sys.path.insert(0, "/opt/trn_rl_repo")
    import concourse.bass as bass

import ml_dtypes
import concourse.bacc as bacc
import concourse.tile as tile
from concourse import mybir
from concourse.bass import IndirectOffsetOnAxis
from concourse.masks import make_identity
from contextlib import ExitStack

F32 = mybir.dt.float32
F32R = mybir.dt.float32r
BF16 = mybir.dt.bfloat16
U32 = mybir.dt.uint32
I32 = mybir.dt.int32
AF = mybir.ActivationFunctionType
ALU = mybir.AluOpType

# problem constants
NCORES = 8
B, T, F = 16, 2048, 80
STK, STRIDE = 4, 4
N = 512                 # frames per batch after subsampling
SF = STK * F            # 320 stacked feature dim
EDIM = 16
V = 8192
D = 512                 # encoder dim
BLOC = B // NCORES      # 2 batches per core
R = BLOC * N            # 1024 rows per core
RT = R // 128           # 8 row tiles
VC = 1024               # vocab chunk for logits
NVC = V // VC           # 8
EPS_LN = 1e-6
MARGIN = 1e-4           # corr count margin (|logits|~2.5, top-2 gaps ~0.1)

_NC_CACHE = {}


def _build_program():
    if "nc" in _NC_CACHE:
        return _NC_CACHE["nc"]
    nc = bacc.Bacc("TRN2", target_bir_lowering=False)

    stack_rows = nc.declare_dram_parameter("stack_rows", [R, SF], F32, isOutput=False)
    stackT = nc.declare_dram_parameter("stackT", [BLOC, SF, N], F32, isOutput=False)
    proj = nc.declare_dram_parameter("proj", [SF, EDIM], F32, isOutput=False)
    projsum = nc.declare_dram_parameter("projsum", [1, EDIM], F32, isOutput=False)
    ek = nc.declare_dram_parameter("Ek", [128, V], BF16, isOutput=False)
    w = nc.declare_dram_parameter("W", [D, V], F32R, isOutput=False)
    wt = nc.declare_dram_parameter("Wt", [V, D], F32, isOutput=False)
    encT = nc.declare_dram_parameter("encT", [BLOC, D, N], F32R, isOutput=False)
    enc_rows = nc.declare_dram_parameter("enc_rows", [BLOC, N, D], F32, isOutput=False)
    maskce = nc.declare_dram_parameter("maskce", [R, 1], F32, isOutput=False)

    out_stats = nc.declare_dram_parameter("out_stats", [1, 8], F32, isOutput=True)
    out_pres = nc.declare_dram_parameter("out_pres", [V, 1], F32, isOutput=True)

    codes_dram = nc.dram_tensor("codes_scratch", [R + 128, 1], U32)

    with tile.TileContext(nc) as tc, ExitStack() as ctx:
        const_p = ctx.enter_context(tc.tile_pool(name="const", bufs=1))
        small_p = ctx.enter_context(tc.tile_pool(name="small", bufs=4))
        cols_p = ctx.enter_context(tc.tile_pool(name="cols", bufs=1))
        stage0_p = ctx.enter_context(tc.tile_pool(name="stage0", bufs=2))
        dots_p = ctx.enter_context(tc.tile_pool(name="dots", bufs=2))
        wpool = ctx.enter_context(tc.tile_pool(name="wpool", bufs=2))
        scr_p = ctx.enter_context(tc.tile_pool(name="scr", bufs=2))
        psum_big = ctx.enter_context(tc.tile_pool(name="psb", bufs=3, space="PSUM"))
        psum_sm = ctx.enter_context(tc.tile_pool(name="pss", bufs=2, space="PSUM"))

        # ---------------- constants / persistent tiles ----------------
        ident = const_p.tile([128, 128], F32)
        make_identity(nc, ident[:])
        eps_t = const_p.tile([128, 1], F32)
        nc.vector.memset(eps_t[:], EPS_LN)
        ones_t = const_p.tile([128, 1], F32)
        nc.vector.memset(ones_t[:], 1.0)
        csum_b = const_p.tile([128, EDIM], F32)
        _ps_ap = projsum[:]
        nc.sync.dma_start(
            csum_b[:],
            bass.AP(tensor=_ps_ap.tensor, offset=_ps_ap.offset,
                    ap=[[0, 128], _ps_ap.ap[-1]]))
        ek_sb = const_p.tile([128, V], BF16)
        nc.sync.dma_start(ek_sb[:], ek[:])
        # two persistent Kahan x tiles (zero padding rows stay zero)
        xk_bufs = []
        for i_ in range(2):
            xkt = const_p.tile([128, 128], BF16, name=f"xk_{i_}")
            nc.vector.memset(xkt[:], 0.0)
            xk_bufs.append(xkt)

        # projection chunks: [128,16] x2 + [64,16]
        pj0 = const_p.tile([128, EDIM], F32)
        pj1 = const_p.tile([128, EDIM], F32)
        pj2 = const_p.tile([64, EDIM], F32)
        nc.sync.dma_start(pj0[:], proj[0:128, :])
        nc.sync.dma_start(pj1[:], proj[128:256, :])
        nc.sync.dma_start(pj2[:], proj[256:320, :])
        pj = [pj0, pj1, pj2]

        # stackT resident per batch: [128,512] x2 + [64,512] each
        st_sb = []
        for b_ in range(BLOC):
            row = []
            for kc, (k0, k1) in enumerate([(0, 128), (128, 256), (256, 320)]):
                t_ = const_p.tile([k1 - k0, N], F32, name=f"stackT_{b_}_{kc}")
                nc.sync.dma_start(t_[:], stackT[b_, k0:k1, :])
                row.append(t_)
            st_sb.append(row)

        # encT resident per batch: 4 chunks [128, 512]
        et_sb = []
        for b_ in range(BLOC):
            row = []
            for kc in range(4):
                t_ = const_p.tile([128, N], F32R, name=f"encT_{b_}_{kc}")
                nc.sync.dma_start(t_[:], encT[b_, kc * 128:(kc + 1) * 128, :])
                row.append(t_)
            et_sb.append(row)

        # persistent per-row-tile column stores
        s_cols = cols_p.tile([128, RT, NVC], F32)      # sum-exp partials
        c_cols = cols_p.tile([128, RT, NVC], F32)      # count partials
        ltgt_cols = cols_p.tile([128, RT], F32)        # target logits
        ethr_cols = cols_p.tile([128, RT], F32)        # exp(tgt+margin)
        mask_cols = cols_p.tile([128, RT], F32)        # CE masks
        red_cols = cols_p.tile([128, 2 * RT], F32)     # masked nll | corr

        # zero the codes scratch padding (indices read past the last tile)
        zpad = const_p.tile([128, 1], U32)
        nc.vector.memset(zpad[:], 0)
        nc.sync.dma_start(codes_dram[R:R + 128, :], zpad[:])

        # ---------------- stage 0 + A: codes per row tile ----------------
        for rt in range(RT):
            b_ = rt // 4
            c0 = (rt % 4) * 128  # frame offset within batch

            stk_t = stage0_p.tile([128, SF], F32, name="stk_t")
            nc.sync.dma_start(stk_t[:], stack_rows[rt * 128:(rt + 1) * 128, :])
            stats = small_p.tile([128, 6], F32, name="stats")
            nc.vector.bn_stats(stats[:], stk_t[:])
            mv = small_p.tile([128, 2], F32, name="mv")
            nc.vector.bn_aggr(mv[:], stats[:])
            rstd = small_p.tile([128, 1], F32, name="rstd")
            nc.scalar.activation(rstd[:], mv[:, 1:2], AF.Sqrt, bias=eps_t[:])
            nc.vector.reciprocal(rstd[:], rstd[:])

            psq = psum_sm.tile([128, EDIM], F32, name="psq", tag="sm")
            for kc, (k0, k1) in enumerate([(0, 128), (128, 256), (256, 320)]):
                nc.tensor.matmul(psq[:], st_sb[b_][kc][:, c0:c0 + 128], pj[kc][:],
                                 start=(kc == 0), stop=(kc == 2))
            mu_c = small_p.tile([128, EDIM], F32, name="mu_c")
            nc.vector.tensor_scalar(mu_c[:], csum_b[:], mv[:, 0:1], None, ALU.mult)
            x_t = small_p.tile([128, EDIM], F32, name="x_t")
            nc.vector.tensor_tensor(out=x_t[:], in0=psq[:], in1=mu_c[:],
                                    op=ALU.subtract)
            nc.vector.tensor_scalar(x_t[:], x_t[:], rstd[:], None, ALU.mult)

            pst = psum_sm.tile([16, 128], F32, name="pst", tag="sm")
            nc.tensor.transpose(pst[:], x_t[:], ident[:])

            xk = xk_bufs[rt % 2]
            xh_f = small_p.tile([16, 128], F32, name="xh_f")
            nc.vector.tensor_copy(xk[0:16, :], pst[:])           # hi (cast)
            nc.vector.tensor_copy(xh_f[:], xk[0:16, :])          # back to f32
            nc.vector.tensor_tensor(out=xh_f[:], in0=pst[:], in1=xh_f[:],
                                    op=ALU.subtract)             # residual
            nc.vector.tensor_copy(xk[32:48, :], xh_f[:])         # lo (cast)
            nc.vector.tensor_copy(xk[64:80, :], xk[0:16, :])     # hi again
            nc.vector.tensor_copy(xk[96:112, :], xk[32:48, :])   # lo again

            dots_sb = dots_p.tile([128, V], F32, name="dots_sb")
            for h in range(8):
                psd = psum_big.tile([128, VC], F32, name="psd", tag="big")
                for j in range(VC // 512):
                    nc.tensor.matmul(
                        psd[:, j * 512:(j + 1) * 512], xk[:],
                        ek_sb[:, h * VC + j * 512:h * VC + (j + 1) * 512],
                        start=True, stop=True)
                nc.scalar.activation(dots_sb[:, h * VC:(h + 1) * VC], psd[:],
                                     AF.Copy)

            m8 = small_p.tile([128, 8], F32, name="m8")
            i8 = small_p.tile([128, 8], U32, name="i8")
            nc.vector.max(m8[:], dots_sb[:])
            nc.vector.max_index(i8[:], m8[:], dots_sb[:])
            nc.sync.dma_start(codes_dram[rt * 128:(rt + 1) * 128, :], i8[:, 0:1])

        # ---------------- per-row-tile CE prep (targets, gather) --------
        tgt_i = []
        for rt in range(RT):
            b_ = rt // 4
            c0 = (rt % 4) * 128
            tgt_t = small_p.tile([128, 1], U32, name=f"tgt_{rt}", bufs=RT)
            nc.sync.dma_start(tgt_t[:], codes_dram[rt * 128 + 1:rt * 128 + 129, :])
            tgt_i.append(tgt_t)

            nc.sync.dma_start(mask_cols[:, rt:rt + 1],
                              maskce[rt * 128:(rt + 1) * 128, :])

            g_t = scr_p.tile([128, D], F32, name="g_t")
            nc.gpsimd.indirect_dma_start(
                out=g_t[:], out_offset=None, in_=wt[:],
                in_offset=IndirectOffsetOnAxis(ap=tgt_t[:, :1], axis=0))
            er_t = scr_p.tile([128, D], F32, name="er_t")
            nc.sync.dma_start(er_t[:], enc_rows[b_, c0:c0 + 128, :])
            prod = scr_p.tile([128, D], F32, name="prod")
            nc.vector.tensor_tensor(out=prod[:], in0=er_t[:], in1=g_t[:],
                                    op=ALU.mult)
            nc.vector.reduce_sum(ltgt_cols[:, rt:rt + 1], prod[:],
                                 axis=mybir.AxisListType.X)
            thr = small_p.tile([128, 1], F32, name="thr")
            nc.vector.tensor_scalar(thr[:], ltgt_cols[:, rt:rt + 1], MARGIN, None,
                                    ALU.add)
            nc.scalar.activation(ethr_cols[:, rt:rt + 1], thr[:], AF.Exp)

        # ---------------- stage B: logits sweep (vc outer) --------------
        for h in range(NVC):
            w_sb = []
            for kc in range(4):
                wt_t = wpool.tile([128, VC], F32R, name=f"w_sb{kc}", tag=f"w{kc}")
                nc.sync.dma_start(wt_t[:], w[kc * 128:(kc + 1) * 128,
                                             h * VC:(h + 1) * VC])
                w_sb.append(wt_t)
            for rt in range(RT):
                b_ = rt // 4
                c0 = (rt % 4) * 128
                psl = psum_big.tile([128, VC], F32, name="psl", tag="big")
                for kc in range(4):
                    for j in range(VC // 512):
                        nc.tensor.matmul(
                            psl[:, j * 512:(j + 1) * 512],
                            et_sb[b_][kc][:, c0:c0 + 128],
                            w_sb[kc][:, j * 512:(j + 1) * 512],
                            start=(kc == 0), stop=(kc == 3))
                exp_t = scr_p.tile([128, VC], F32, name="exp_t")
                nc.scalar.activation(exp_t[:], psl[:], AF.Exp,
                                     accum_out=s_cols[:, rt, h:h + 1])
                gt_t = scr_p.tile([128, VC], F32, name="gt_t")
                nc.vector.tensor_scalar(gt_t[:], exp_t[:],
                                        ethr_cols[:, rt:rt + 1], None,
                                        ALU.is_gt, ALU.add,
                                        accum_out=c_cols[:, rt, h:h + 1])

        # ---------------- finalize per row tile -------------------------
        for rt in range(RT):
            s_t = small_p.tile([128, 1], F32, name="s_t")
            nc.vector.reduce_sum(s_t[:], s_cols[:, rt, :], axis=mybir.AxisListType.X)
            cnt_t = small_p.tile([128, 1], F32, name="cnt_t")
            nc.vector.reduce_sum(cnt_t[:], c_cols[:, rt, :], axis=mybir.AxisListType.X)
            lnS = small_p.tile([128, 1], F32, name="lnS")
            nc.scalar.activation(lnS[:], s_t[:], AF.Ln)
            nll = small_p.tile([128, 1], F32, name="nll")
            nc.vector.tensor_tensor(out=nll[:], in0=lnS[:],
                                    in1=ltgt_cols[:, rt:rt + 1], op=ALU.subtract)
            nc.vector.tensor_tensor(out=red_cols[:, rt:rt + 1], in0=nll[:],
                                    in1=mask_cols[:, rt:rt + 1], op=ALU.mult)
            corr = small_p.tile([128, 1], F32, name="corr")
            nc.vector.tensor_scalar(corr[:], cnt_t[:], 0.5, None, ALU.is_lt)
            nc.vector.tensor_tensor(out=red_cols[:, RT + rt:RT + rt + 1],
                                    in0=corr[:], in1=mask_cols[:, rt:rt + 1],
                                    op=ALU.mult)

            # presence scatter: idx = code * mask (masked rows -> 0), skip t=511
            pidx_f = small_p.tile([128, 1], F32, name="pidx_f")
            nc.vector.tensor_copy(pidx_f[:], tgt_i[rt][:, :1])
            nc.vector.tensor_tensor(out=pidx_f[:], in0=pidx_f[:],
                                    in1=mask_cols[:, rt:rt + 1], op=ALU.mult)
            pidx = small_p.tile([128, 1], I32, name="pidx")
            nc.vector.tensor_copy(pidx[:], pidx_f[:])
            np_ = 127 if rt % 4 == 3 else 128
            nc.gpsimd.indirect_dma_start(
                out=out_pres[:], out_offset=IndirectOffsetOnAxis(
                    ap=pidx[:np_, :1], axis=0),
                in_=ones_t[:np_, :], in_offset=None)

        # ---------------- partition reduction ---------------------------
        psr = psum_sm.tile([1, 2 * RT], F32, name="psr", tag="sm")
        nc.tensor.matmul(psr[:], ones_t[:], red_cols[:], start=True, stop=True)
        fin = small_p.tile([1, 8], F32, name="fin")
        nc.vector.reduce_sum(fin[:, 0:1], psr[0:1, 0:RT], axis=mybir.AxisListType.X)
        nc.vector.reduce_sum(fin[:, 1:2], psr[0:1, RT:2 * RT],
                             axis=mybir.AxisListType.X)
        nc.vector.memset(fin[:, 2:8], 0.0)
        nc.sync.dma_start(out_stats[:], fin[:])

    nc.compile()
    _NC_CACHE["nc"] = nc
    return nc


def _prep_core_inputs(inputs, core):
    feats = inputs["feats"]
    lengths = inputs["feats_lengths"]
    enc = inputs["encoder_out"]
    proj = inputs["projection"]
    emb = inputs["embeddings"]
    top = inputs["top_n_out"]

    b0 = core * BLOC
    fb = np.ascontiguousarray(feats[b0:b0 + BLOC]).reshape(BLOC, N, SF)
    stack_rows = np.ascontiguousarray(fb.reshape(R, SF), dtype=np.float32)
    stackT = np.ascontiguousarray(fb.transpose(0, 2, 1), dtype=np.float32)

    encb = enc[b0:b0 + BLOC]
    enc_rows = np.ascontiguousarray(encb, dtype=np.float32)
    encT = np.ascontiguousarray(encb.transpose(0, 2, 1), dtype=np.float32)

    L = (lengths[b0:b0 + BLOC].astype(np.int64) // STRIDE)
    t_idx = np.arange(N)
    maskce = (t_idx[None, :] < (L[:, None] - 1)).astype(np.float32).reshape(R, 1)

    return {
        "stack_rows": stack_rows,
        "stackT": stackT,
        "encT": encT,
        "enc_rows": enc_rows,
        "maskce": np.ascontiguousarray(maskce),
    }


def _prep_shared_inputs(inputs):
    proj = np.asarray(inputs["projection"], dtype=np.float32)
    emb = np.asarray(inputs["embeddings"], dtype=np.float32)
    top = np.asarray(inputs["top_n_out"], dtype=np.float32)

    projsum = proj.sum(0, keepdims=True).astype(np.float32)  # [1, 16]

    Et = np.ascontiguousarray(emb[:, 0, :].T, dtype=np.float32)  # [16, V]
    Eh = Et.astype(ml_dtypes.bfloat16).astype(np.float32)
    El = (Et - Eh).astype(ml_dtypes.bfloat16).astype(np.float32)
    Z = np.zeros_like(Eh)
    # row pairing with x tile [xh;0;xl;0;xh;0;xl;0]: hh + lh + hl + ll
    Ek = np.concatenate(
        [Eh, Z, Eh, Z, El, Z, El, Z], axis=0).astype(ml_dtypes.bfloat16)

    W = np.ascontiguousarray(top[0, 0], dtype=np.float32)        # [D, V]
    Wt = np.ascontiguousarray(W.T)                               # [V, D]
    return {
        "proj": np.ascontiguousarray(proj),
        "projsum": projsum,
        "Ek": np.ascontiguousarray(Ek),
        "W": W,
        "Wt": Wt,
    }


def _combine(results, inputs):
    lengths = np.asarray(inputs["feats_lengths"]).astype(np.int64)
    L = lengths // STRIDE
    num_codes = float((L - 1).sum())

    nll_sum = 0.0
    corr_sum = 0.0
    pres = np.zeros(V, dtype=bool)
    for r in results:
        st = np.asarray(r["out_stats"]).reshape(-1)
        nll_sum += float(st[0])
        corr_sum += float(st[1])
        pres |= np.asarray(r["out_pres"]).reshape(-1) > 0.0

    loss = np.float32(nll_sum / num_codes)
    acc = np.float32(corr_sum / num_codes)
    uniq = np.float32(pres.sum())
    return np.array([loss, acc, np.float32(num_codes), uniq], dtype=np.float32)


def _run(inputs, trace=False):
    from concourse.bass_utils import run_bass_kernel_spmd
    nc = _build_program()
    shared = _prep_shared_inputs(inputs)
    in_maps = []
    for core in range(NCORES):
        m = dict(shared)
        m.update(_prep_core_inputs(inputs, core))
        in_maps.append(m)
    res = run_bass_kernel_spmd(nc, in_maps, core_ids=list(range(NCORES)),
                               trace=trace)
    out = _combine(res.results, inputs)
    return out, res


def _run_sim(inputs, core=0):
    """Single-core simulator run (correctness debugging)."""
    from concourse.bass_interp import CoreSim
    nc = _build_program()
    m = dict(_prep_shared_inputs(inputs))
    m.update(_prep_core_inputs(inputs, core))
    sim = CoreSim(nc)
    for k, v in m.items():
        sim.tensor(k)[:] = v
    sim.simulate()
    return {k: np.array(sim.tensor(k)) for k in ("out_stats", "out_pres")}


def kernel(**inputs) -> np.ndarray:
    out, _ = _run(inputs, trace=False)
    return out


# revision 10
# speedup vs baseline: 1.1556x; 1.1556x over previous
"""Trainium2 Bass kernel for nn_NestRQModel (NEST-RQ pretraining loss).

Strategy: pure data-parallel over batch (2 batches per core, 8 cores), no
collectives.  Each core computes partial masked sums (nll, corr) and a
vocab-presence bitmap; the host combines them into the 4 scalar outputs.

Per-core pipeline (1024 rows = 2 batches x 512 frames):
  stage 0: LayerNorm stats (bn_stats) on stacked frames [128,320/tile];
           projection matmul q = stack @ P with LN folded in afterwards as a
           per-row affine fix  x = s*(q - mu*colsum(P))  (LN commutes through
           the linear projection).  Row-norm of x is skipped entirely: codes
           are an argmax over cosine-monotone scores, and the e2 term of the
           reference distance varies by < 1e-8 across the codebook.
  stage A: dots = x @ E^T as ONE K=64 matmul per tile via bf16 hi/lo Kahan
           stacking (error ~1e-6, full PE rate); argmax via DVE max/max_index.
  stage B: logits = enc @ W in float32r (full PE rate, N=512);
           ACT Exp with accum_out -> row sum-of-exp (no max subtraction
           needed: |logits| < ~4 so exp cannot overflow);
           corr  = [#(exp(logits) > exp(tgt+margin)) == 0] via one 2x-mode
           tensor_scalar is_gt with accum;
           tgt logit = indirect-DMA gather of W^T rows + fused TT-reduce dot.
  presence: indirect-DMA scatter of 1.0 at (code*mask) indices — identical
           semantics to reference's presence[masked_tgt]=1 (masked rows
           scatter index 0).
"""
import os
import sys

import numpy as np

os.environ.setdefault("MYCRO_LOCAL_CACHE", "1")

try:
    import concourse.bass as bass
except ImportError:
    sys.path.insert(0, "/opt/trn_rl_repo")
    import concourse.bass as bass

import ml_dtypes
import concourse.bacc as bacc
import concourse.tile as tile
from concourse import mybir
from concourse.bass import IndirectOffsetOnAxis
from concourse.masks import make_identity
from contextlib import ExitStack

F32 = mybir.dt.float32
F32R = mybir.dt.float32r
BF16 = mybir.dt.bfloat16
U32 = mybir.dt.uint32
I32 = mybir.dt.int32
AF = mybir.ActivationFunctionType
ALU = mybir.AluOpType

# problem constants
NCORES = 8
B, T, F = 16, 2048, 80
STK, STRIDE = 4, 4
N = 512                 # frames per batch after subsampling
SF = STK * F            # 320 stacked feature dim
EDIM = 16
V = 8192
D = 512                 # encoder dim
BLOC = B // NCORES      # 2 batches per core
R = BLOC * N            # 1024 rows per core
RT = R // 128           # 8 row tiles
VC = 1024               # vocab chunk for logits
NVC = V // VC           # 8
EPS_LN = 1e-6
MARGIN = 5e-3           # corr margin: covers bf16 matmul noise (~1.5e-3)

_NC_CACHE = {}


def _build_program():
    if "nc" in _NC_CACHE:
        return _NC_CACHE["nc"]
    nc = bacc.Bacc("TRN2", target_bir_lowering=False)

    stack_rows = nc.declare_dram_parameter("stack_rows", [R, SF], F32, isOutput=False)
    stackT = nc.declare_dram_parameter("stackT", [BLOC, SF, N], F32, isOutput=False)
    proj = nc.declare_dram_parameter("proj", [SF, EDIM], F32, isOutput=False)
    projsum = nc.declare_dram_parameter("projsum", [1, EDIM], F32, isOutput=False)
    ek = nc.declare_dram_parameter("Ek", [128, V], BF16, isOutput=False)
    w = nc.declare_dram_parameter("W", [D, V], BF16, isOutput=False)
    wt = nc.declare_dram_parameter("Wt", [V, D], F32, isOutput=False)
    encT = nc.declare_dram_parameter("encT", [BLOC, D, N], BF16, isOutput=False)
    enc_rows = nc.declare_dram_parameter("enc_rows", [BLOC, N, D], F32, isOutput=False)
    maskce = nc.declare_dram_parameter("maskce", [R, 1], F32, isOutput=False)

    out_stats = nc.declare_dram_parameter("out_stats", [1, 8], F32, isOutput=True)
    out_pres = nc.declare_dram_parameter("out_pres", [V, 1], F32, isOutput=True)

    codes_dram = nc.dram_tensor("codes_scratch", [R + 128, 1], U32)

    with tile.TileContext(nc) as tc, ExitStack() as ctx:
        const_p = ctx.enter_context(tc.tile_pool(name="const", bufs=1))
        small_p = ctx.enter_context(tc.tile_pool(name="small", bufs=4))
        cols_p = ctx.enter_context(tc.tile_pool(name="cols", bufs=1))
        stage0_p = ctx.enter_context(tc.tile_pool(name="stage0", bufs=2))
        dots_p = ctx.enter_context(tc.tile_pool(name="dots", bufs=2))
        wpool = ctx.enter_context(tc.tile_pool(name="wpool", bufs=2))
        scr_p = ctx.enter_context(tc.tile_pool(name="scr", bufs=2))
        psum_big = ctx.enter_context(tc.tile_pool(name="psb", bufs=2, space="PSUM"))
        psum_sm = ctx.enter_context(tc.tile_pool(name="pss", bufs=4, space="PSUM"))

        # ---------------- constants / persistent tiles ----------------
        ident = const_p.tile([128, 128], F32)
        make_identity(nc, ident[:])
        eps_t = const_p.tile([128, 1], F32)
        nc.vector.memset(eps_t[:], EPS_LN)
        ones_t = const_p.tile([128, 1], F32)
        nc.vector.memset(ones_t[:], 1.0)
        csum_b = const_p.tile([128, EDIM], F32)
        _ps_ap = projsum[:]
        nc.sync.dma_start(
            csum_b[:],
            bass.AP(tensor=_ps_ap.tensor, offset=_ps_ap.offset,
                    ap=[[0, 128], _ps_ap.ap[-1]]))
        ek_sb = const_p.tile([128, V], BF16)
        nc.sync.dma_start(ek_sb[:], ek[:])
        # two persistent Kahan x tiles (zero padding rows stay zero)
        xk_bufs = []
        for i_ in range(RT):
            xkt = const_p.tile([128, 128], BF16, name=f"xk_{i_}")
            nc.vector.memset(xkt[:], 0.0)
            xk_bufs.append(xkt)

        # projection chunks: [128,16] x2 + [64,16]
        pj0 = const_p.tile([128, EDIM], F32)
        pj1 = const_p.tile([128, EDIM], F32)
        pj2 = const_p.tile([64, EDIM], F32)
        nc.sync.dma_start(pj0[:], proj[0:128, :])
        nc.sync.dma_start(pj1[:], proj[128:256, :])
        nc.sync.dma_start(pj2[:], proj[256:320, :])
        pj = [pj0, pj1, pj2]

        # stackT resident per batch: [128,512] x2 + [64,512] each
        st_sb = []
        for b_ in range(BLOC):
            row = []
            for kc, (k0, k1) in enumerate([(0, 128), (128, 256), (256, 320)]):
                t_ = const_p.tile([k1 - k0, N], F32, name=f"stackT_{b_}_{kc}")
                nc.sync.dma_start(t_[:], stackT[b_, k0:k1, :])
                row.append(t_)
            st_sb.append(row)

        # encT resident per batch: 4 chunks [128, 512]
        et_sb = []
        for b_ in range(BLOC):
            row = []
            for kc in range(4):
                t_ = const_p.tile([128, N], BF16, name=f"encT_{b_}_{kc}")
                nc.sync.dma_start(t_[:], encT[b_, kc * 128:(kc + 1) * 128, :])
                row.append(t_)
            et_sb.append(row)

        # persistent per-row-tile column stores
        s_cols = cols_p.tile([128, RT, NVC], F32)      # sum-exp partials
        c_cols = cols_p.tile([128, RT, NVC], F32)      # count partials
        ltgt_cols = cols_p.tile([128, RT], F32)        # target logits
        ethr_cols = cols_p.tile([128, RT], F32)        # exp(tgt+margin)
        mask_cols = cols_p.tile([128, RT], F32)        # CE masks
        red_cols = cols_p.tile([128, 2 * RT], F32)     # masked nll | corr

        # zero the codes scratch padding (indices read past the last tile)
        zpad = const_p.tile([128, 1], U32)
        nc.vector.memset(zpad[:], 0)
        nc.sync.dma_start(codes_dram[R:R + 128, :], zpad[:])

        # ---------------- stage 0: x Kahan tiles for all row tiles ------
        for rt in range(RT):
            b_ = rt // 4
            c0 = (rt % 4) * 128  # frame offset within batch

            stk_t = stage0_p.tile([128, SF], F32, name="stk_t")
            nc.sync.dma_start(stk_t[:], stack_rows[rt * 128:(rt + 1) * 128, :])
            stats = small_p.tile([128, 6], F32, name="stats")
            nc.vector.bn_stats(stats[:], stk_t[:])
            mv = small_p.tile([128, 2], F32, name="mv")
            nc.vector.bn_aggr(mv[:], stats[:])
            rstd = small_p.tile([128, 1], F32, name="rstd")
            nc.scalar.activation(rstd[:], mv[:, 1:2], AF.Sqrt, bias=eps_t[:])
            nc.vector.reciprocal(rstd[:], rstd[:])

            psq = psum_sm.tile([128, EDIM], F32, name="psq", tag="sm")
            for kc, (k0, k1) in enumerate([(0, 128), (128, 256), (256, 320)]):
                nc.tensor.matmul(psq[:], st_sb[b_][kc][:, c0:c0 + 128], pj[kc][:],
                                 start=(kc == 0), stop=(kc == 2))
            mu_c = small_p.tile([128, EDIM], F32, name="mu_c")
            nc.vector.tensor_scalar(mu_c[:], csum_b[:], mv[:, 0:1], None, ALU.mult)
            x_t = small_p.tile([128, EDIM], F32, name="x_t")
            nc.vector.tensor_tensor(out=x_t[:], in0=psq[:], in1=mu_c[:],
                                    op=ALU.subtract)
            nc.vector.tensor_scalar(x_t[:], x_t[:], rstd[:], None, ALU.mult)

            pst = psum_sm.tile([16, 128], F32, name="pst", tag="sm")
            nc.tensor.transpose(pst[:], x_t[:], ident[:])

            xk = xk_bufs[rt]
            xh_f = small_p.tile([16, 128], F32, name="xh_f")
            nc.vector.tensor_copy(xk[0:16, :], pst[:])           # hi (cast)
            nc.vector.tensor_copy(xh_f[:], xk[0:16, :])          # back to f32
            nc.vector.tensor_tensor(out=xh_f[:], in0=pst[:], in1=xh_f[:],
                                    op=ALU.subtract)             # residual
            nc.vector.tensor_copy(xk[32:48, :], xh_f[:])         # lo (cast)
            nc.vector.tensor_copy(xk[64:80, :], xk[0:16, :])     # hi again
            nc.vector.tensor_copy(xk[96:112, :], xk[32:48, :])   # lo again

        # ---------------- stage A: dots + argmax per row tile -----------
        for rt in range(RT):
            xk = xk_bufs[rt]
            dots_sb = dots_p.tile([128, V], F32, name="dots_sb")
            for h in range(8):
                psd = psum_big.tile([128, VC], F32, name="psd", tag="big")
                for j in range(VC // 512):
                    nc.tensor.matmul(
                        psd[:, j * 512:(j + 1) * 512], xk[:],
                        ek_sb[:, h * VC + j * 512:h * VC + (j + 1) * 512],
                        start=True, stop=True)
                nc.scalar.activation(dots_sb[:, h * VC:(h + 1) * VC], psd[:],
                                     AF.Copy)

            m8 = small_p.tile([128, 8], F32, name="m8")
            i8 = small_p.tile([128, 8], U32, name="i8")
            nc.vector.max(m8[:], dots_sb[:])
            nc.vector.max_index(i8[:], m8[:], dots_sb[:])
            nc.sync.dma_start(codes_dram[rt * 128:(rt + 1) * 128, :], i8[:, 0:1])

        # ---------------- per-row-tile CE prep (targets, gather) --------
        tgt_i = []
        for rt in range(RT):
            b_ = rt // 4
            c0 = (rt % 4) * 128
            tgt_t = small_p.tile([128, 1], U32, name=f"tgt_{rt}", bufs=RT)
            nc.sync.dma_start(tgt_t[:], codes_dram[rt * 128 + 1:rt * 128 + 129, :])
            tgt_i.append(tgt_t)

            nc.sync.dma_start(mask_cols[:, rt:rt + 1],
                              maskce[rt * 128:(rt + 1) * 128, :])

            g_t = scr_p.tile([128, D], F32, name="g_t")
            nc.gpsimd.indirect_dma_start(
                out=g_t[:], out_offset=None, in_=wt[:],
                in_offset=IndirectOffsetOnAxis(ap=tgt_t[:, :1], axis=0))
            er_t = scr_p.tile([128, D], F32, name="er_t")
            nc.sync.dma_start(er_t[:], enc_rows[b_, c0:c0 + 128, :])
            prod = scr_p.tile([128, D], F32, name="prod")
            nc.vector.tensor_tensor(out=prod[:], in0=er_t[:], in1=g_t[:],
                                    op=ALU.mult)
            nc.vector.reduce_sum(ltgt_cols[:, rt:rt + 1], prod[:],
                                 axis=mybir.AxisListType.X)
            thr = small_p.tile([128, 1], F32, name="thr")
            nc.vector.tensor_scalar(thr[:], ltgt_cols[:, rt:rt + 1], MARGIN, None,
                                    ALU.add)
            nc.scalar.activation(ethr_cols[:, rt:rt + 1], thr[:], AF.Exp)

        # ---------------- stage B: logits sweep (vc outer) --------------
        for h in range(NVC):
            w_sb = []
            for kc in range(4):
                wt_t = wpool.tile([128, VC], BF16, name=f"w_sb{kc}", tag=f"w{kc}")
                nc.sync.dma_start(wt_t[:], w[kc * 128:(kc + 1) * 128,
                                             h * VC:(h + 1) * VC])
                w_sb.append(wt_t)
            for rt in range(RT):
                b_ = rt // 4
                c0 = (rt % 4) * 128
                psl = psum_big.tile([128, VC], F32, name="psl", tag="big")
                for kc in range(4):
                    for j in range(VC // 512):
                        nc.tensor.matmul(
                            psl[:, j * 512:(j + 1) * 512],
                            et_sb[b_][kc][:, c0:c0 + 128],
                            w_sb[kc][:, j * 512:(j + 1) * 512],
                            start=(kc == 0), stop=(kc == 3))
                exp_t = scr_p.tile([128, VC], BF16, name="exp_t")
                nc.scalar.activation(exp_t[:], psl[:], AF.Exp,
                                     accum_out=s_cols[:, rt, h:h + 1])
                gt_t = scr_p.tile([128, VC], BF16, name="gt_t")
                nc.vector.tensor_scalar(gt_t[:], exp_t[:],
                                        ethr_cols[:, rt:rt + 1], None,
                                        ALU.is_gt, ALU.add,
                                        accum_out=c_cols[:, rt, h:h + 1])

        # ---------------- finalize per row tile -------------------------
        for rt in range(RT):
            s_t = small_p.tile([128, 1], F32, name="s_t")
            nc.vector.reduce_sum(s_t[:], s_cols[:, rt, :], axis=mybir.AxisListType.X)
            cnt_t = small_p.tile([128, 1], F32, name="cnt_t")
            nc.vector.reduce_sum(cnt_t[:], c_cols[:, rt, :], axis=mybir.AxisListType.X)
            lnS = small_p.tile([128, 1], F32, name="lnS")
            nc.scalar.activation(lnS[:], s_t[:], AF.Ln)
            nll = small_p.tile([128, 1], F32, name="nll")
            nc.vector.tensor_tensor(out=nll[:], in0=lnS[:],
                                    in1=ltgt_cols[:, rt:rt + 1], op=ALU.subtract)
            nc.vector.tensor_tensor(out=red_cols[:, rt:rt + 1], in0=nll[:],
                                    in1=mask_cols[:, rt:rt + 1], op=ALU.mult)
            corr = small_p.tile([128, 1], F32, name="corr")
            nc.vector.tensor_scalar(corr[:], cnt_t[:], 0.5, None, ALU.is_lt)
            nc.vector.tensor_tensor(out=red_cols[:, RT + rt:RT + rt + 1],
                                    in0=corr[:], in1=mask_cols[:, rt:rt + 1],
                                    op=ALU.mult)

            # presence scatter: idx = code * mask (masked rows -> 0), skip t=511
            pidx_f = small_p.tile([128, 1], F32, name="pidx_f")
            nc.vector.tensor_copy(pidx_f[:], tgt_i[rt][:, :1])
            nc.vector.tensor_tensor(out=pidx_f[:], in0=pidx_f[:],
                                    in1=mask_cols[:, rt:rt + 1], op=ALU.mult)
            pidx = small_p.tile([128, 1], I32, name="pidx")
            nc.vector.tensor_copy(pidx[:], pidx_f[:])
            np_ = 127 if rt % 4 == 3 else 128
            nc.gpsimd.indirect_dma_start(
                out=out_pres[:], out_offset=IndirectOffsetOnAxis(
                    ap=pidx[:np_, :1], axis=0),
                in_=ones_t[:np_, :], in_offset=None)

        # ---------------- partition reduction ---------------------------
        psr = psum_sm.tile([1, 2 * RT], F32, name="psr", tag="sm")
        nc.tensor.matmul(psr[:], ones_t[:], red_cols[:], start=True, stop=True)
        fin = small_p.tile([1, 8], F32, name="fin")
        nc.vector.reduce_sum(fin[:, 0:1], psr[0:1, 0:RT], axis=mybir.AxisListType.X)
        nc.vector.reduce_sum(fin[:, 1:2], psr[0:1, RT:2 * RT],
                             axis=mybir.AxisListType.X)
        nc.vector.memset(fin[:, 2:8], 0.0)
        nc.sync.dma_start(out_stats[:], fin[:])

    nc.compile()
    _NC_CACHE["nc"] = nc
    return nc


def _prep_core_inputs(inputs, core):
    feats = inputs["feats"]
    lengths = inputs["feats_lengths"]
    enc = inputs["encoder_out"]
    proj = inputs["projection"]
    emb = inputs["embeddings"]
    top = inputs["top_n_out"]

    b0 = core * BLOC
    fb = np.ascontiguousarray(feats[b0:b0 + BLOC]).reshape(BLOC, N, SF)
    stack_rows = np.ascontiguousarray(fb.reshape(R, SF), dtype=np.float32)
    stackT = np.ascontiguousarray(fb.transpose(0, 2, 1), dtype=np.float32)

    encb = enc[b0:b0 + BLOC]
    enc_rows = np.ascontiguousarray(encb, dtype=np.float32)
    encT = np.ascontiguousarray(
        encb.transpose(0, 2, 1).astype(ml_dtypes.bfloat16))

    L = (lengths[b0:b0 + BLOC].astype(np.int64) // STRIDE)
    t_idx = np.arange(N)
    maskce = (t_idx[None, :] < (L[:, None] - 1)).astype(np.float32).reshape(R, 1)

    return {
        "stack_rows": stack_rows,
        "stackT": stackT,
        "encT": encT,
        "enc_rows": enc_rows,
        "maskce": np.ascontiguousarray(maskce),
    }


def _prep_shared_inputs(inputs):
    proj = np.asarray(inputs["projection"], dtype=np.float32)
    emb = np.asarray(inputs["embeddings"], dtype=np.float32)
    top = np.asarray(inputs["top_n_out"], dtype=np.float32)

    projsum = proj.sum(0, keepdims=True).astype(np.float32)  # [1, 16]

    Et = np.ascontiguousarray(emb[:, 0, :].T, dtype=np.float32)  # [16, V]
    Eh = Et.astype(ml_dtypes.bfloat16).astype(np.float32)
    El = (Et - Eh).astype(ml_dtypes.bfloat16).astype(np.float32)
    Z = np.zeros_like(Eh)
    # row pairing with x tile [xh;0;xl;0;xh;0;xl;0]: hh + lh + hl + ll
    Ek = np.concatenate(
        [Eh, Z, Eh, Z, El, Z, El, Z], axis=0).astype(ml_dtypes.bfloat16)

    W = np.ascontiguousarray(top[0, 0], dtype=np.float32)        # [D, V]
    Wt = np.ascontiguousarray(W.T)                               # [V, D]
    return {
        "proj": np.ascontiguousarray(proj),
        "projsum": projsum,
        "Ek": np.ascontiguousarray(Ek),
        "W": np.ascontiguousarray(W.astype(ml_dtypes.bfloat16)),
        "Wt": Wt,
    }


def _combine(results, inputs):
    lengths = np.asarray(inputs["feats_lengths"]).astype(np.int64)
    L = lengths // STRIDE
    num_codes = float((L - 1).sum())

    nll_sum = 0.0
    corr_sum = 0.0
    pres = np.zeros(V, dtype=bool)
    for r in results:
        st = np.asarray(r["out_stats"]).reshape(-1)
        nll_sum += float(st[0])
        corr_sum += float(st[1])
        pres |= np.asarray(r["out_pres"]).reshape(-1) > 0.0

    loss = np.float32(nll_sum / num_codes)
    acc = np.float32(corr_sum / num_codes)
    uniq = np.float32(pres.sum())
    return np.array([loss, acc, np.float32(num_codes), uniq], dtype=np.float32)


def _run(inputs, trace=False):
    from concourse.bass_utils import run_bass_kernel_spmd
    nc = _build_program()
    shared = _prep_shared_inputs(inputs)
    in_maps = []
    for core in range(NCORES):
        m = dict(shared)
        m.update(_prep_core_inputs(inputs, core))
        in_maps.append(m)
    res = run_bass_kernel_spmd(nc, in_maps, core_ids=list(range(NCORES)),
                               trace=trace)
    out = _combine(res.results, inputs)
    return out, res


def _run_sim(inputs, core=0):
    """Single-core simulator run (correctness debugging)."""
    from concourse.bass_interp import CoreSim
    nc = _build_program()
    m = dict(_prep_shared_inputs(inputs))
    m.update(_prep_core_inputs(inputs, core))
    sim = CoreSim(nc)
    for k, v in m.items():
        sim.tensor(k)[:] = v
    sim.simulate()
    return {k: np.array(sim.tensor(k)) for k in ("out_stats", "out_pres")}


def kernel(**inputs) -> np.ndarray:
    out, _ = _run(inputs, trace=False)
    return out
